# revision 3
# baseline (speedup 1.0000x reference)
"""PointPillarScatter kernel for 8 Trainium2 NeuronCores.

Strategy (data-parallel over batch, one core per batch element):
  host:   fold BN into the 64x64 linear; per batch: drop invalid pillars,
          sort by spatial index, upload xT (transposed, +ones row), per-chunk
          one-hot position vectors and row-gather indices as input tensors.
  device: MLP via PE matmul (h = relu(xT_aug.T @ W_aug)), bounce h through
          DRAM, indirect-DMA row-gather into per-chunk padded tiles (64 rows
          per 256-position output chunk, two chunks per 128-partition tile),
          then a one-hot "scatter matmul" per chunk pair:
             out[64ch, 256pos] = h_rows.T @ S,   S[k, n] = (pos[k] == n)
          which does scatter + transpose + zero-fill in one PE op. Dense
          [128, 256] results are copied PSUM->SBUF and written with large
          grouped DMAs.

The program structure is input-independent; all data-dependent placement
lives in input tensors (gidx, pos).
"""

import sys
import numpy as np

sys.path.insert(0, "/opt/trn_rl_repo")

import jax  # noqa: E402
from jax.sharding import Mesh, PartitionSpec  # noqa: E402
from jax.experimental.shard_map import shard_map  # noqa: E402

import concourse.bass as bass  # noqa: E402
import concourse.bacc as bacc  # noqa: E402
import concourse.mybir as mybir  # noqa: E402
from concourse import tile  # noqa: E402
from concourse import bass2jax  # noqa: E402

F32 = mybir.dt.float32
U32 = mybir.dt.uint32

B = 8
P_PER = 16384
C = 64
NX = 432
NY = 496
NSLOT = NX * NY          # 214272
NCH = 256                # output positions per chunk
NCHUNK = NSLOT // NCH    # 837
NPAIR = (NCHUNK + 1) // 2  # 419 (last pair has a dummy upper chunk)
RPC = 64                 # padded pillar rows per chunk
GRP = 11                 # chunk pairs per output DMA group (418 = 38*11)
HROWS = P_PER + 128      # h table rows in DRAM (incl. zero rows)
ZROW = P_PER             # index of a guaranteed-zero row
EPS = 1e-5

_cache = {}


def _build_program():
    nc = bacc.Bacc(None, target_bir_lowering=False, debug=False)

    xT = nc.dram_tensor("xT", [C + 1, P_PER], F32, kind="ExternalInput")
    w = nc.dram_tensor("w", [C + 1, C], F32, kind="ExternalInput")
    gidx = nc.dram_tensor("gidx", [128, NPAIR], U32, kind="ExternalInput")
    pos = nc.dram_tensor("pos", [128, NPAIR], F32, kind="ExternalInput")
    out = nc.dram_tensor("out", [C, NSLOT], F32, kind="ExternalOutput")
    hdram = nc.dram_tensor("hdram", [HROWS, C], F32)

    with tile.TileContext(nc) as tc:
        with (
            tc.tile_pool(name="const", bufs=1) as cpool,
            tc.tile_pool(name="xin", bufs=1) as xpool,
            tc.tile_pool(name="hbuf", bufs=1) as hpool,
            tc.tile_pool(name="mlp_ps", bufs=2, space="PSUM") as mlp_ps,
            tc.tile_pool(name="scat_ps", bufs=5, space="PSUM") as scat_ps,
            tc.tile_pool(name="spool", bufs=6) as spool,
            tc.tile_pool(name="gpool", bufs=6) as gpool,
            tc.tile_pool(name="opool", bufs=2) as opool,
        ):
            # ---- constants / inputs to SBUF
            w_sb = cpool.tile([C + 1, C], F32)
            nc.sync.dma_start(w_sb[:], w.ap())
            gidx_sb = cpool.tile([128, NPAIR], U32)
            nc.sync.dma_start(gidx_sb[:], gidx.ap())
            pos_sb = cpool.tile([128, NPAIR], F32)
            nc.sync.dma_start(pos_sb[:], pos.ap())
            iota_sb = cpool.tile([128, NCH], F32)
            nc.gpsimd.iota(
                iota_sb[:], pattern=[[1, NCH]], base=0, channel_multiplier=0,
                allow_small_or_imprecise_dtypes=True,
            )
            zt = cpool.tile([128, C], F32)
            nc.vector.memset(zt[:], 0.0)

            xT_sb = xpool.tile([C + 1, P_PER], F32)
            for i in range(4):
                nc.sync.dma_start(
                    xT_sb[:, bass.ts(i, P_PER // 4)],
                    xT.ap()[:, bass.ts(i, P_PER // 4)],
                )

            # ---- MLP: h[128t+p, :] = relu(x @ Wf + bf)
            h_sb = hpool.tile([128, (P_PER // 128) * C], F32)
            for t in range(P_PER // 128):
                ps = mlp_ps.tile([128, C], F32)
                nc.tensor.matmul(
                    ps[:], lhsT=xT_sb[:, bass.ts(t, 128)], rhs=w_sb[:],
                    start=True, stop=True,
                )
                nc.scalar.activation(
                    h_sb[:, bass.ts(t, C)], ps[:],
                    mybir.ActivationFunctionType.Relu,
                )

            # ---- h -> DRAM (row r = 128t + p), zero the pad rows
            hd_main = hdram.ap()[0:P_PER, :].rearrange("(t p) c -> p t c", p=128)
            nc.sync.dma_start(hd_main, h_sb[:])
            hd_pad = hdram.ap()[P_PER:HROWS, :].rearrange("(t p) c -> p t c", p=128)
            nc.sync.dma_start(hd_pad, zt[:])

            # ---- scatter: one chunk pair per iteration
            ob = None
            for t in range(NPAIR):
                pt = gpool.tile([128, C], F32)
                nc.gpsimd.indirect_dma_start(
                    out=pt[:],
                    out_offset=None,
                    in_=hdram.ap(),
                    in_offset=bass.IndirectOffsetOnAxis(
                        ap=gidx_sb[:, t:t + 1], axis=0
                    ),
                )
                st = spool.tile([128, NCH], F32)
                nc.vector.tensor_scalar(
                    st[:], iota_sb[:], pos_sb[:, t:t + 1], None,
                    op0=mybir.AluOpType.is_equal,
                )
                ps = scat_ps.tile([128, NCH], F32)
                nc.tensor.matmul(
                    ps[0:64, :], lhsT=pt[0:64, :], rhs=st[0:64, :],
                    start=True, stop=True, tile_position=(0, 0),
                )
                nc.tensor.matmul(
                    ps[64:128, :], lhsT=pt[64:128, :], rhs=st[64:128, :],
                    start=True, stop=True, tile_position=(64, 64),
                )

                j = t % GRP
                if j == 0:
                    ob = opool.tile([128, GRP * NCH], F32)
                nc.vector.tensor_copy(ob[:, bass.ts(j, NCH)], ps[:])

                if t == NPAIR - 1:
                    # final pair: only the lower half (chunk 836) exists
                    dst = out.ap()[:, NCHUNK * NCH - NCH:]
                    nc.sync.dma_start(dst, ob[0:64, 0:NCH])
                elif j == GRP - 1:
                    g = t // GRP
                    span = out.ap()[:, g * GRP * 2 * NCH:(g + 1) * GRP * 2 * NCH]
                    span = span.rearrange("c (j x) -> c j x", j=GRP)
                    nc.sync.dma_start(span[:, :, 0:NCH], ob[0:64, :])
                    nc.sync.dma_start(span[:, :, NCH:2 * NCH], ob[64:128, :])

    nc.compile()
    return nc


class _Runner:
    """Compile-once executor for the SPMD program on 8 cores."""

    def __init__(self, nc):
        self.nc = nc
        bass2jax.install_neuronx_cc_hook()
        part_name = (nc.partition_id_tensor.name
                     if nc.partition_id_tensor else None)
        in_names, out_names, out_avals = [], [], []
        for alloc in nc.m.functions[0].allocations:
            if not isinstance(alloc, mybir.MemoryLocationSet):
                continue
            name = alloc.memorylocations[0].name
            if alloc.kind == "ExternalInput":
                if name != part_name:
                    in_names.append(name)
            elif alloc.kind == "ExternalOutput":
                out_names.append(name)
                out_avals.append(jax.core.ShapedArray(
                    tuple(alloc.tensor_shape), mybir.dt.np(alloc.dtype)))
        self.in_names = in_names
        self.out_names = out_names
        self.out_avals = out_avals
        n_io = len(in_names) + len(out_names)

        devices = jax.devices()[:B]
        self.mesh = Mesh(np.asarray(devices), ("core",))
        all_in_names = list(in_names) + list(out_names)
        if part_name is not None:
            all_in_names.append(part_name)
        all_in_names = tuple(all_in_names)

        def _body(*args):
            operands = list(args)
            if part_name is not None:
                operands.append(bass2jax.partition_id_tensor())
            outs = bass2jax._bass_exec_p.bind(
                *operands,
                out_avals=tuple(out_avals),
                in_names=all_in_names,
                out_names=tuple(out_names),
                lowering_input_output_aliases=(),
                sim_require_finite=True,
                sim_require_nnan=True,
                nc=nc,
            )
            return tuple(outs)

        self.fn = jax.jit(
            shard_map(
                _body, mesh=self.mesh,
                in_specs=(PartitionSpec("core"),) * n_io,
                out_specs=(PartitionSpec("core"),) * len(out_names),
                check_rep=False,
            ),
            keep_unused=True,
        )
        # persistent pre-zeroed "output seed" buffers (kernel writes every
        # element, so their contents are never observed)
        self.zero_outs = [
            self.to_device(np.zeros((B * a.shape[0], *a.shape[1:]), a.dtype))
            for a in out_avals
        ]

    def to_device(self, arr):
        from jax.sharding import NamedSharding
        return jax.device_put(arr, NamedSharding(self.mesh, PartitionSpec("core")))

    def concat_inputs(self, in_maps):
        return [
            np.concatenate([m[name] for m in in_maps], axis=0)
            for name in self.in_names
        ]

    def run(self, dev_or_np_inputs):
        return self.fn(*dev_or_np_inputs, *self.zero_outs)

    def run_maps(self, in_maps, fetch=True):
        outs = self.run(self.concat_inputs(in_maps))
        if not fetch:
            jax.block_until_ready(outs)
            return None
        res = []
        for c in range(B):
            res.append({
                name: np.asarray(outs[i]).reshape(
                    B, *self.out_avals[i].shape)[c]
                for i, name in enumerate(self.out_names)
            })
        return res


def _get_runner():
    if "runner" not in _cache:
        _cache["runner"] = _Runner(_build_program())
    return _cache["runner"]


def _host_prep(pillar_features, voxel_coords, topk_w, topk_b, bn_gamma,
               bn_beta, bn_mean, bn_var):
    """Build per-core input maps."""
    s = (bn_gamma / np.sqrt(bn_var + EPS)).astype(np.float32)
    wf = (topk_w * s[None, :]).astype(np.float32)
    bf = ((topk_b - bn_mean) * s + bn_beta).astype(np.float32)
    w_aug = np.concatenate([wf, bf[None, :]], axis=0)  # [65, 64]

    in_maps = []
    for b in range(B):
        sl = slice(b * P_PER, (b + 1) * P_PER)
        cb = voxel_coords[sl]
        xb = pillar_features[sl]
        valid = cb[:, 4] != -1
        g = (cb[valid, 1] + cb[valid, 2] * NX + cb[valid, 3]).astype(np.int64)
        xv = xb[valid]
        order = np.argsort(g, kind="stable")
        g = g[order]
        xv = xv[order]
        nv = g.shape[0]

        xTa = np.zeros((C + 1, P_PER), np.float32)
        xTa[:C, :nv] = xv.T
        xTa[C, :] = 1.0

        # chunk runs
        bounds = np.searchsorted(g, np.arange(NCHUNK + 1) * NCH)
        ncs = np.diff(bounds)
        if ncs.max() > RPC:
            raise RuntimeError(
                f"chunk with {ncs.max()} pillars exceeds capacity {RPC}"
            )

        gidx = np.full((128, NPAIR), ZROW, np.uint32)
        posm = np.full((128, NPAIR), -1.0, np.float32)
        # partition p of pair tile t handles chunk 2t (p<64) or 2t+1 (p>=64),
        # row j = p % 64 within the chunk
        for c in range(NCHUNK):
            lo, hi = bounds[c], bounds[c + 1]
            n = hi - lo
            if n == 0:
                continue
            t = c // 2
            p0 = (c % 2) * RPC
            gidx[p0:p0 + n, t] = np.arange(lo, hi, dtype=np.uint32)
            posm[p0:p0 + n, t] = (g[lo:hi] - c * NCH).astype(np.float32)

        in_maps.append({"xT": xTa, "w": w_aug, "gidx": gidx, "pos": posm})
    return in_maps


def kernel(**inputs):
    pillar_features = np.asarray(inputs["pillar_features"], np.float32)
    voxel_coords = np.asarray(inputs["voxel_coords"])
    topk_w = np.asarray(inputs["topk_w"], np.float32)
    topk_b = np.asarray(inputs["topk_b"], np.float32)
    bn_gamma = np.asarray(inputs["bn_gamma"], np.float32)
    bn_beta = np.asarray(inputs["bn_beta"], np.float32)
    bn_mean = np.asarray(inputs["bn_mean"], np.float32)
    bn_var = np.asarray(inputs["bn_var"], np.float32)
    assert int(np.asarray(inputs["batch_size"])) == B
    assert int(np.asarray(inputs["nx"])) == NX
    assert int(np.asarray(inputs["ny"])) == NY

    in_maps = _host_prep(pillar_features, voxel_coords, topk_w, topk_b,
                         bn_gamma, bn_beta, bn_mean, bn_var)
    res = _get_runner().run_maps(in_maps)
    out = np.stack([res[b]["out"] for b in range(B)], axis=0)
    return out.reshape(B, C, NY, NX)
